# revision 1
# baseline (speedup 1.0000x reference)
"""Multi-head causal attention (B=2,T=2048,C=1024,H=16,Dh=64) on 8 trn2 cores.

Sharding: tensor-parallel over heads — core c owns heads (2c, 2c+1).
Per core: QKV projections for its 128 q/k/v columns, causal flash attention
for its 2 heads x 2 batches, partial output projection with its 128 rows of
Wp. Host sums the 8 partial projections and adds the bias.

Dataflow is "transposed": activations live as [feature, token] so every
matmul contracts along partitions. Softmax runs without max-subtraction
(scores are O(5) for this problem family), sums come free via a ones-column
appended to v, and normalization happens on the [64, T] head outputs rather
than the [T, T] weights.
"""
import numpy as np
import ml_dtypes

import concourse.bass as bass
import concourse.mybir as mybir
import concourse.tile as tile
from concourse.bass_utils import run_bass_kernel_spmd
from concourse.masks import make_identity
from concourse.vector_clock import ScopedClock

BF16 = mybir.dt.bfloat16
F32 = mybir.dt.float32

B, T, C = 2, 2048, 1024
H, DH = 16, 64
NCORES = 8
HPC = 128  # head-columns per core (2 heads x 64)
NI = 512   # query-strip width
NJ = 128   # key-tile width
NSTRIP = T // NI          # 4 strips per (b)
NJT = T // NJ             # 16 j-tiles per (b)
NCC = C // 128            # 8 contraction chunks
SCALE = DH ** -0.5


class TileContextP(tile.TileContext):
    """This walrus build caps sync waits at 1 per instruction (2 for
    EventSemaphore). Tile can emit more. Legalize by spilling excess waits
    onto same-engine nops emitted just before the instruction, and do the
    same for the kernel-tail drain."""

    def _commit_instruction(self, inst, lazy_reg_writes: bool = True):
        si = getattr(inst, "sync_info", None)
        if si is not None and si.on_wait:
            cap = 2 if isinstance(inst, mybir.InstEventSemaphore) else 1
            if len(si.on_wait) > cap:
                waits = list(si.on_wait)
                keep, spill = waits[:cap - 1] if cap > 1 else [], waits[cap - 1:]
                # keep the last wait on the inst, spill the rest
                spill, last = spill[:-1], spill[-1:]
                for w in spill:
                    nop = mybir.InstNoOp(
                        name=self.nc.get_next_instruction_name(),
                        engine=inst.engine,
                        sync_info=mybir.SyncInfo(on_wait=[w], on_update=[]),
                        bass_nofuse=True,
                    )
                    self._add_instruction(nop)
                si.on_wait = keep + last
        return super()._commit_instruction(inst, lazy_reg_writes)

    def _drain_and_barrier(self, tick_clock, wait_clock):
        probe = self.nc.sync.nop()
        wait_clock.add_sem_waits(
            probe.ins, ScopedClock({None: tick_clock.global_clock})
        )
        waits = list(probe.ins.sync_info.on_wait) if probe.ins.sync_info else []
        if probe.ins.sync_info:
            probe.ins.sync_info.on_wait = []
        for w in waits:
            n = self.nc.sync.nop()
            si = n.ins.sync_info
            if si is None:
                n.ins.sync_info = mybir.SyncInfo(on_wait=[w], on_update=[])
            else:
                si.on_wait = [w]
        self.nc.sync.drain()
        self.nc.all_engine_barrier()
        assert self.sems is not None
        popped = self.nc._tile_sem_poison_stack.pop()
        assert popped is self._sem_poison
        self.nc.clear_and_free_semaphores(list(self.sems.allocated().values()))
        self.nc.all_engine_barrier()


def build_nc():
    nc = bass.Bass()
    xT_h = nc.dram_tensor("xT", [B, C, T], BF16, kind="ExternalInput")
    wq_h = nc.dram_tensor("wq", [C, HPC], BF16, kind="ExternalInput")
    wk_h = nc.dram_tensor("wk", [C, HPC], BF16, kind="ExternalInput")
    wv_h = nc.dram_tensor("wv", [C, HPC], BF16, kind="ExternalInput")
    wp_h = nc.dram_tensor("wp", [C, C], BF16, kind="ExternalInput")
    mk_h = nc.dram_tensor("masks", [4, NJ, NI], BF16, kind="ExternalInput")
    y_h = nc.dram_tensor("y_out", [C, NI], F32, kind="ExternalOutput")
    rsc_h = nc.dram_tensor("rscratch", [B * NSTRIP, 2 * NI], F32)
    # per-batch reshard buffers: shard j carries this core's head-pair rows for
    # batch-b tokens [256j, 256j+256)
    a2a_in = [nc.dram_tensor(f"a2a_in{b}", [NCORES, NJ, NI // 2], BF16)
              for b in range(B)]
    a2a_out = [nc.dram_tensor(f"a2a_out{b}", [NCORES, NJ, NI // 2], BF16)
               for b in range(B)]

    with TileContextP(nc) as tc, \
         tc.tile_pool(name="singles", bufs=1) as singles, \
         tc.tile_pool(name="xtp", bufs=2) as xtp, \
         tc.tile_pool(name="qkv", bufs=2) as qkvp, \
         tc.tile_pool(name="vaugp", bufs=2) as vaugp, \
         tc.tile_pool(name="weip", bufs=5) as weip, \
         tc.tile_pool(name="attp", bufs=8) as attp, \
         tc.tile_pool(name="smallp", bufs=2) as smallp, \
         tc.tile_pool(name="ydr", bufs=4) as ydr, \
         tc.tile_pool(name="bigps", bufs=2, space="PSUM") as bigps, \
         tc.tile_pool(name="auxps", bufs=2, space="PSUM") as auxps:

        # ---- constants / weights (qkv weights first — they gate the first matmuls)
        wq = singles.tile([128, NCC, HPC], BF16)
        wk = singles.tile([128, NCC, HPC], BF16)
        wv = singles.tile([128, NCC, HPC], BF16)
        for w_t, w_hh in ((wq, wq_h), (wk, wk_h), (wv, wv_h)):
            wsrc = w_hh.rearrange("(n p) m -> p n m", p=128)
            for q in range(4):
                nc.scalar.dma_start(out=w_t[:, 2 * q:2 * q + 2, :],
                                    in_=wsrc[:, 2 * q:2 * q + 2, :])
        ident = singles.tile([128, 128], BF16)
        make_identity(nc, ident)
        masks = singles.tile([128, 4, NI], BF16)
        wp = singles.tile([128, NCC, C], BF16)

        proj_jobs = []
        for b in range(B):
            xt = xtp.tile([128, NCC, T], BF16)
            xsrc = xT_h[b].rearrange("(n p) t -> p n t", p=128)
            for cc in range(NCC):  # split loads: latency + queue parallelism
                for hx in range(2):
                    nc.sync.dma_start(out=xt[:, cc, hx * 1024:(hx + 1) * 1024],
                                      in_=xsrc[:, cc, hx * 1024:(hx + 1) * 1024])
            if b == 0:
                # masks needed at the first diagonal tile (~strip 0)
                nc.scalar.dma_start(out=masks, in_=mk_h.rearrange("d p i -> p d i"))
            else:
                # wp only needed at projection time
                nc.scalar.dma_start(out=wp, in_=wp_h.rearrange("(n p) m -> p n m", p=128))

            # ---- QKV projections: out[feat, tok] accumulated over NCC chunks
            qt = qkvp.tile([128, T], BF16)
            kt = qkvp.tile([128, T], BF16)
            vt = qkvp.tile([128, T], BF16)
            for w_t, dst in ((wq, qt), (wk, kt), (wv, vt)):
                for ts in range(2):  # 1024-wide psum tiles, two 512 matmuls each
                    ps = bigps.tile([128, 1024], F32, tag="big")
                    for cc in range(NCC):
                        for hf in range(2):
                            nc.tensor.matmul(
                                ps[:, hf * 512:(hf + 1) * 512],
                                w_t[:, cc, :],
                                xt[:, cc, ts * 1024 + hf * 512:ts * 1024 + (hf + 1) * 512],
                                start=(cc == 0), stop=(cc == NCC - 1),
                            )
                    nc.vector.tensor_copy(dst[:, ts * 1024:(ts + 1) * 1024], ps)

            # ---- v -> token-major with ones column:
            # vaug[:, jt, 0:65] = [v_h0 | 1], vaug[:, jt, 65:130] = [v_h1 | 1]
            vaug = vaugp.tile([128, NJT, 130], BF16)
            nc.vector.memset(vaug, 1.0)
            for jt in range(NJT):
                ptr = auxps.tile([128, 128], BF16, tag="aux")
                nc.tensor.transpose(ptr, vt[:, jt * 128:(jt + 1) * 128], ident)
                nc.vector.tensor_copy(vaug[:, jt, 0:64], ptr[:, 0:64])
                nc.vector.tensor_copy(vaug[:, jt, 65:129], ptr[:, 64:128])

            # ---- causal flash attention, both heads packed
            for st in range(NSTRIP):
                i0 = st * NI
                njt = 4 * (st + 1)
                oaug = auxps.tile([65, 2 * NI], F32, tag="aux")
                for jt in range(njt):
                    j0 = jt * NJ
                    # diagonal tiles only touch columns i_local >= d*128
                    d = jt - (njt - 4)
                    lo = max(d, 0) * 128
                    sco = bigps.tile([128, 1024], F32, tag="big")
                    for h in range(2):
                        nc.tensor.matmul(
                            sco[:, h * NI + lo:(h + 1) * NI],
                            kt[h * 64:(h + 1) * 64, j0:j0 + NJ],
                            qt[h * 64:(h + 1) * 64, i0 + lo:i0 + NI],
                            start=True, stop=True,
                        )
                    wei = weip.tile([128, 2 * NI], BF16)
                    if d < 1:
                        # off-diagonal and d==0 (lo=0): one full-width exp is
                        # cheaper than two subrange ops
                        nc.scalar.activation(wei, sco,
                                             mybir.ActivationFunctionType.Exp,
                                             scale=SCALE)
                    else:
                        for h in range(2):
                            nc.scalar.activation(
                                wei[:, h * NI + lo:(h + 1) * NI],
                                sco[:, h * NI + lo:(h + 1) * NI],
                                mybir.ActivationFunctionType.Exp, scale=SCALE,
                            )
                    if d >= 0:
                        # causal mask for every diagonal tile (incl. d==0)
                        for h in range(2):
                            nc.vector.tensor_mul(
                                wei[:, h * NI + lo:(h + 1) * NI],
                                wei[:, h * NI + lo:(h + 1) * NI],
                                masks[:, d, lo:],
                            )
                    for h in range(2):
                        nc.tensor.matmul(
                            oaug[:, h * NI + lo:(h + 1) * NI],
                            vaug[:, jt, h * 65:(h + 1) * 65],
                            wei[:, h * NI + lo:(h + 1) * NI],
                            start=(jt == 0), stop=(jt == njt - 1),
                        )

                # ---- normalize: att[h*64+d, i] = oaug[d, i] / oaug[64, i]
                att = attp.tile([128, NI], BF16)
                sl = b * NSTRIP + st
                r = smallp.tile([1, 2 * NI], F32, tag="r")
                nc.vector.reciprocal(r, oaug[64:65, :])
                nc.gpsimd.dma_start(out=rsc_h[sl:sl + 1, :], in_=r)
                rb = smallp.tile([64, 2 * NI], F32, tag="rb")
                bcast = bass.AP(
                    tensor=rsc_h.tensor if hasattr(rsc_h, "tensor") else rsc_h,
                    offset=sl * 2 * NI,
                    ap=[[0, 64], [1, 2 * NI]],
                )
                nc.gpsimd.dma_start(out=rb, in_=bcast)
                for h in range(2):
                    nc.vector.tensor_mul(
                        att[h * 64:(h + 1) * 64, :],
                        oaug[0:64, h * NI:(h + 1) * NI],
                        rb[:, h * NI:(h + 1) * NI],
                    )
                # ship this strip's two 256-token halves to their reshard slots
                # (Pool queue: keeps SP free for the projection input loads)
                for hf in range(2):
                    nc.gpsimd.dma_start(
                        out=a2a_in[b][2 * st + hf],
                        in_=att[:, hf * 256:(hf + 1) * 256],
                    )

            # ---- projection for the PREVIOUS batch, emitted before this
            # batch's collective: after attention in the PE stream (no
            # head-of-line block) and before the next collective in program
            # order (instructions emitted after a collective get serialized
            # behind it)
            for pb in proj_jobs:
                _emit_proj(nc, pb, a2a_out, attp, bigps, ydr, wp, y_h)
            proj_jobs.clear()

            # ---- all-to-all for this batch: heads-sharded -> token-sharded.
            # Batch 0's reshard overlaps batch 1's attention.
            nc.gpsimd.collective_compute(
                "AllToAll",
                mybir.AluOpType.bypass,
                replica_groups=[list(range(NCORES))],
                ins=[a2a_in[b][:, :, :].opt()],
                outs=[a2a_out[b][:, :, :].opt()],
            )
            proj_jobs.append(b)

        for pb in proj_jobs:
            _emit_proj(nc, pb, a2a_out, attp, bigps, ydr, wp, y_h)
    return nc


def _emit_proj(nc, b, a2a_out, attp, bigps, ydr, wp, y_h):
    rhs_tiles = []
    for j in range(NCORES):
        rt_ = attp.tile([128, NI // 2], BF16, tag="prhs")
        nc.sync.dma_start(out=rt_, in_=a2a_out[b][j])
        rhs_tiles.append(rt_)
    for nt in range(8):
        py = bigps.tile([128, NI // 2], F32, tag="big")
        for j in range(NCORES):
            nc.tensor.matmul(py, wp[:, j, nt * 128:(nt + 1) * 128],
                             rhs_tiles[j],
                             start=(j == 0), stop=(j == NCORES - 1))
        yo = ydr.tile([128, NI // 2], F32)
        nc.scalar.copy(yo, py)
        nc.sync.dma_start(
            out=y_h[nt * 128:(nt + 1) * 128, b * 256:(b + 1) * 256],
            in_=yo,
        )


_NC_CACHE = {}


def _get_nc():
    if "nc" not in _NC_CACHE:
        _NC_CACHE["nc"] = build_nc()
    return _NC_CACHE["nc"]


def _host_masks():
    jl = np.arange(NJ)[:, None]
    il = np.arange(NI)[None, :]
    return np.stack([(il >= jl + d * 128) for d in range(4)]).astype(ml_dtypes.bfloat16)


def kernel(x, Wk, Wq, Wv, Wp, bp):
    x = np.asarray(x)
    xT = np.ascontiguousarray(x.transpose(0, 2, 1)).astype(ml_dtypes.bfloat16)
    wpb = np.asarray(Wp).astype(ml_dtypes.bfloat16)
    masks = _host_masks()
    in_maps = []
    for c in range(NCORES):
        cs = slice(c * HPC, (c + 1) * HPC)
        in_maps.append({
            "xT": xT,
            "wq": np.ascontiguousarray(Wq[:, cs]).astype(ml_dtypes.bfloat16),
            "wk": np.ascontiguousarray(Wk[:, cs]).astype(ml_dtypes.bfloat16),
            "wv": np.ascontiguousarray(Wv[:, cs]).astype(ml_dtypes.bfloat16),
            "wp": wpb,
            "masks": masks,
        })
    res = run_bass_kernel_spmd(_get_nc(), in_maps, list(range(NCORES)))
    # core c's y_out[:, b*256:(b+1)*256] covers batch-b tokens [256c, 256c+256)
    yT = np.zeros((B, C, T), np.float32)
    for c in range(NCORES):
        yo = res.results[c]["y_out"]
        for b in range(B):
            yT[b, :, 256 * c:256 * (c + 1)] = yo[:, b * 256:(b + 1) * 256]
    y = yT.transpose(0, 2, 1) + np.asarray(bp)[None, None, :]
    return np.ascontiguousarray(y, dtype=np.float32)

